# revision 13
# baseline (speedup 1.0000x reference)
"""Bass/Tile TRN2 kernel for additive-attention pooling.

Math per sample s:
    e = tanh(x[s] @ W + b)          # (T, 1)
    a = softmax(e, axis=0)          # over T
    y[s] = sum_t a[t] * x[s, t, :]  # (U,)

tanh is bounded in (-1, 1), so softmax needs no max-subtraction:
    p = exp(e);  y[s] = (sum_t p[t] x[s,t]) / (sum_t p[t])

Sharding: data-parallel over batch across 8 NeuronCores (32 samples each).

Per-core dataflow. x is streamed once, in "superchunks" of Q*128 timesteps
laid out q-packed (t = sc*Q*128 + p*Q + q) so each SBUF partition receives
Q*2KiB contiguous from HBM in one large DMA (few DMA-issue instructions,
big packets). Per superchunk:
  - DVE scalar_tensor_tensor: e_col = sum_u (x * W) per 128x512 slice
    (single fused pass over x, 1 elem/lane/cyc)
  - DVE adds bias b; ACT tanh; ACT exp with accum_out row sums
  - PE weighted sum, 4-way column-tiled: slice q -> col group q%4, fp32
    matmuls in distinct col groups stream via separate XBUSes (~4x conc.)
Per sample: partial rows {0,32,64,96} combined via a selector matmul,
denominator via ones-matmul, DVE reciprocal, ACT scaled copy, DMA out.
"""

from contextlib import ExitStack

import numpy as np

B, T, U = 256, 2048, 512
N_CORES = 8
B_LOC = B // N_CORES
P = 128

_BUILD_CACHE = {}


def _emit(ctx, tc, x, W, b, y, xbufs):
    from concourse import mybir

    nc = tc.nc
    f32 = mybir.dt.float32
    Alu = mybir.AluOpType
    Act = mybir.ActivationFunctionType

    b_loc, t_len, u = x.shape
    tch = t_len // P          # 128-timestep chunks
    Q = 8 if tch % 8 == 0 else 4   # chunks per superchunk
    nsc = tch // Q

    const = ctx.enter_context(tc.tile_pool(name="const", bufs=1))
    xp = ctx.enter_context(tc.tile_pool(name="xp", bufs=xbufs))
    scr_p = ctx.enter_context(tc.tile_pool(name="scr", bufs=3))
    ep = ctx.enter_context(tc.tile_pool(name="ep", bufs=6))
    sp = ctx.enter_context(tc.tile_pool(name="sp", bufs=8))
    op = ctx.enter_context(tc.tile_pool(name="op", bufs=4))
    ps_wb = ctx.enter_context(tc.tile_pool(name="ps_wb", bufs=1, space="PSUM"))
    ps_w = ctx.enter_context(tc.tile_pool(name="ps_w", bufs=4, space="PSUM"))
    ps_s = ctx.enter_context(tc.tile_pool(name="ps_s", bufs=2, space="PSUM"))

    # ---- constants ----
    # W as a [1, U] row, broadcast to all 128 partitions via a K=1 matmul.
    w_row = const.tile([1, u], f32)
    nc.sync.dma_start(w_row[:], W.rearrange("u o -> o u"))
    ones_row = const.tile([1, P], f32)
    nc.vector.memset(ones_row[:], 1.0)
    ones_col = const.tile([P, 1], f32)
    nc.vector.memset(ones_col[:], 1.0)
    # selector: 1.0 at partitions {0,32,64,96} — sums the 4 col-group partials
    sel = const.tile([P, 1], f32)
    nc.vector.memset(sel[:], 0.0)
    for j in range(4):
        nc.vector.memset(sel[32 * j:32 * j + 1, :], 1.0)
    wb_ps = ps_wb.tile([P, u], f32, tag="wb_ps")
    nc.tensor.matmul(wb_ps[:], ones_row[:], w_row[:], start=True, stop=True)
    Wb = const.tile([P, u], f32)
    nc.vector.tensor_copy(Wb[:], wb_ps[:])
    # b rearranged to [partition, (sc q)] matching the q-packed x layout
    bt3 = const.tile([P, nsc, Q], f32)
    nc.sync.dma_start(bt3[:], b.rearrange("(sc p q) o -> p sc (q o)", p=P, q=Q))

    xr = x.rearrange("s (sc p q) u -> s sc p (q u)", p=P, q=Q)

    for s in range(b_loc):
        wsum = ps_w.tile([P, u], f32, tag="wsum")
        nc.scalar.memzero(wsum[:])
        rs = sp.tile([P, nsc], f32, tag="rs")
        for sc in range(nsc):
            # fat tile: Q*128 timesteps, partition p holds Q contiguous
            # HBM rows -> one Q*256KiB DMA with Q*2KiB packets
            xt = xp.tile([P, Q * u], f32)
            nc.sync.dma_start(xt[:], xr[s, sc])
            e_sc = ep.tile([P, Q], f32, tag="e_sc")
            for q in range(Q):
                scr = scr_p.tile([P, u], f32)
                nc.vector.scalar_tensor_tensor(
                    out=scr[:],
                    in0=xt[:, q * u:(q + 1) * u],
                    scalar=1.0,
                    in1=Wb[:],
                    op0=Alu.mult,
                    op1=Alu.mult,
                    accum_out=e_sc[:, q:q + 1],
                )
            eb_sc = ep.tile([P, Q], f32, tag="eb_sc")
            nc.vector.tensor_add(eb_sc[:], e_sc[:], bt3[:, sc, :])
            th_sc = ep.tile([P, Q], f32, tag="th_sc")
            nc.scalar.activation(th_sc[:], eb_sc[:], Act.Tanh)
            p_sc = ep.tile([P, Q], f32, tag="p_sc")
            nc.scalar.activation(p_sc[:], th_sc[:], Act.Exp,
                                 accum_out=rs[:, sc:sc + 1])
            # weighted sum: slice q -> col group q%4, partial at psum row 32j
            for q in range(Q):
                c = sc * Q + q
                j = q % 4
                nc.tensor.matmul(
                    wsum[32 * j:32 * j + 1, :],
                    p_sc[:, q:q + 1],
                    xt[:, q * u:(q + 1) * u],
                    start=(c < 4), stop=(c >= tch - 4),
                    tile_position=(0, 32 * j),
                )

        # denominator: rstot[p] = sum_sc rs[p,sc]; s = rstot.T @ ones
        rsd = sp.tile([P, nsc], f32, tag="rsd")
        rstot = sp.tile([P, 1], f32, tag="rstot")
        nc.scalar.activation(rsd[:], rs[:], Act.Copy, accum_out=rstot[:])
        s_ps = ps_s.tile([1, 1], f32)
        nc.tensor.matmul(s_ps[:], rstot[:], ones_col[:], start=True, stop=True)
        inv = sp.tile([1, 1], f32, tag="inv")
        nc.vector.reciprocal(inv[:], s_ps[:])

        # combine the 4 partial rows: copy bank to SBUF, then sel.T @ rows
        wsb = op.tile([P, u], f32, tag="wsb")
        nc.scalar.activation(wsb[:], wsum[:], Act.Copy)
        nc.tensor.matmul(wsum[0:1, :], sel[:], wsb[:], start=True, stop=True)

        orow = op.tile([1, u], f32, tag="orow")
        nc.scalar.activation(orow[:], wsum[0:1, :], Act.Copy, scale=inv[:])
        nc.sync.dma_start(y[s:s + 1, :], orow[:])


def build_nc(b_loc=B_LOC, t_len=T, u=U, xbufs=10):
    key = (b_loc, t_len, u, xbufs)
    if key in _BUILD_CACHE:
        return _BUILD_CACHE[key]
    import concourse.bacc as bacc
    import concourse.tile as tile
    from concourse import mybir

    nc = bacc.Bacc(
        "TRN2",
        target_bir_lowering=False,
        debug=False,
        num_devices=N_CORES,
    )
    x = nc.dram_tensor("x", [b_loc, t_len, u], mybir.dt.float32, kind="ExternalInput").ap()
    W = nc.dram_tensor("W", [u, 1], mybir.dt.float32, kind="ExternalInput").ap()
    b = nc.dram_tensor("b", [t_len, 1], mybir.dt.float32, kind="ExternalInput").ap()
    y = nc.dram_tensor("y", [b_loc, u], mybir.dt.float32, kind="ExternalOutput").ap()

    with tile.TileContext(nc) as tc:
        with ExitStack() as ctx:
            _emit(ctx, tc, x, W, b, y, xbufs)
    nc.compile()
    _BUILD_CACHE[key] = nc
    return nc


def kernel(x, W, b):
    x = np.ascontiguousarray(np.asarray(x, dtype=np.float32))
    W = np.ascontiguousarray(np.asarray(W, dtype=np.float32))
    b = np.ascontiguousarray(np.asarray(b, dtype=np.float32))
    assert x.shape == (B, T, U), x.shape

    from concourse.bass_utils import run_bass_kernel_spmd

    nc = build_nc()
    in_maps = [
        {
            "x": np.ascontiguousarray(x[i * B_LOC:(i + 1) * B_LOC]),
            "W": W,
            "b": b,
        }
        for i in range(N_CORES)
    ]
    res = run_bass_kernel_spmd(nc, in_maps, core_ids=list(range(N_CORES)))
    return np.concatenate([r["y"] for r in res.results], axis=0)
